# revision 1
# baseline (speedup 1.0000x reference)
"""Trainium2 Bass kernel for nn_NormalizedDistanceLoss.

Math: for x in R^{N x D}, with sq_i = ||x_i||^2, the strict-upper-triangle
sum of pairwise squared distances collapses algebraically:

    sum_{i<j} (sq_i + sq_j - 2 x_i.x_j) = N * S - ||s||^2

where S = sum_i sq_i and s = sum_i x_i (column sums).  So the loss

    loss = sum_masked_dist / (sqrt(max_i sq_i) * N(N-1)/2)

needs only one pass over x: per-row squared norms (for S and the max)
and column sums (for s).  Each of the 8 cores reduces its 1024-row
block; the host combines tiny per-core partials.

The input is staged to device DRAM as bf16 (host-side cast), halving
the HBM stream vs f32.  The resulting loss error is ~1e-5 relative:
rowsq picks up ~0.03% error (averaged squaring noise), and the
||s||^2 term it feeds is only ~1.2e-4 of N*S.  Far below the 2e-2
gate.

Per-core device kernel (block = 1024 x 512 bf16 as 8 row-tiles of
[128, 512]; partition p holds DRAM rows p*8..p*8+7, 8KB contiguous),
written as a raw two-block Bass program (no TileContext) with manual
semaphores so the input DMAs issue as the very first instructions:

  - Main block: the three chunked input DMAs ([3,3,2] tiles; sync,
    scalar, sync HWDGE rings) issue immediately — ahead of the const
    memsets and the ACT table load that a TileContext prologue would
    serialize in front of them.  Each chunk is 128 descriptors and
    costs ~1.9us of per-ring descriptor processing regardless of size,
    so chunk count per ring is what matters, not chunk bytes.
    Meanwhile 8 free-dim-512 warmup matmuls on a zeroed tile keep the
    PE HAM activity window hot so the real matmuls run at 2.4 GHz
    instead of the cold 1.2 GHz.
  - Second block: row squared norms as fused square+row-sum per tile
    (6 tiles on DVE scalar_tensor_tensor, 2 on ACT Square, each with
    accum_out); column sums as per-tile ones-vector matmuls on the
    otherwise-idle PE (tiles 0-5 accumulate in one PSUM bank, 6-7 in a
    second, so the first bank's PSUM->SBUF copy overlaps the tail);
    outputs (rowsq [128,8] f32, colsum 2x[512] f32) leave on separate
    rings so their HBM write receipts overlap.
  - The NEFF executes more than once per invocation (PJRT warmup), so
    gpsimd waits for both output DMAs and clears every semaphore at
    the end; without this the next execution's waits all pass
    instantly and compute races the input stream.
"""

import contextlib
import sys

if "/opt/trn_rl_repo" not in sys.path:
    sys.path.insert(0, "/opt/trn_rl_repo")

import numpy as np

try:
    from ml_dtypes import bfloat16 as _bf16_np
except ImportError:  # jax bundles ml_dtypes
    from jax.numpy import bfloat16 as _bf16_np

from concourse import bacc, mybir

N = 8192
D = 512
NCORES = 8
ROWS = N // NCORES  # 1024 rows per core
P = 128
T = ROWS // P  # 8 row-tiles of [128, 512]
WARMUP_MMS = 8

_nc_cache = []


def _build_nc():
    f32 = mybir.dt.float32
    bf16 = mybir.dt.bfloat16
    mult = mybir.AluOpType.mult
    nc = bacc.Bacc(
        "TRN2",
        target_bir_lowering=False,
        debug=False,
        num_devices=NCORES,
    )
    x_dram = nc.dram_tensor("x_blk", [ROWS, D], bf16, kind="ExternalInput")
    rowsq_dram = nc.dram_tensor("rowsq", [P, T], f32, kind="ExternalOutput")
    colsum_dram = nc.dram_tensor("colsum", [1, 2 * D], f32, kind="ExternalOutput")

    es = contextlib.ExitStack()
    X = es.enter_context(nc.sbuf_tensor("X", [P, T, D], bf16))
    ones = es.enter_context(nc.sbuf_tensor("ones", [P, 1], bf16))
    zerob = es.enter_context(nc.sbuf_tensor("zerob", [P, 1], f32))
    wrhs = es.enter_context(nc.sbuf_tensor("wrhs", [P, D], bf16))
    xsq_v = es.enter_context(nc.sbuf_tensor("xsq_v", [P, D], bf16))
    xsq_a = es.enter_context(nc.sbuf_tensor("xsq_a", [P, D], bf16))
    rowsq = es.enter_context(nc.sbuf_tensor("rowsq_sb", [P, T], f32))
    cs = es.enter_context(nc.sbuf_tensor("cs_sb", [1, 2 * D], f32))
    ps0 = nc.alloc_psum_tensor("ps0", [1, D], f32)
    ps1 = nc.alloc_psum_tensor("ps1", [1, D], f32)
    psw = nc.alloc_psum_tensor("psw", [1, D], f32)

    s_const = es.enter_context(nc.semaphore("s_const"))
    s_c0 = es.enter_context(nc.semaphore("s_c0"))
    s_c1 = es.enter_context(nc.semaphore("s_c1"))
    s_c2 = es.enter_context(nc.semaphore("s_c2"))
    s_sqp = es.enter_context(nc.semaphore("s_sqp"))
    s_sq = es.enter_context(nc.semaphore("s_sq"))
    s_pe0 = es.enter_context(nc.semaphore("s_pe0"))
    s_pe1 = es.enter_context(nc.semaphore("s_pe1"))
    s_out0 = es.enter_context(nc.semaphore("s_out0"))
    s_out1 = es.enter_context(nc.semaphore("s_out1"))

    x_r = x_dram[:].rearrange("(p t) d -> p t d", p=P)

    # ---- main block: input DMAs first, then consts and PE warmups ----
    nc.sync.dma_start(X[:, 0:3, :], x_r[:, 0:3, :]).then_inc(s_c0, 16)
    nc.sync.dma_start(X[:, 6:8, :], x_r[:, 6:8, :]).then_inc(s_c2, 16)
    nc.scalar.dma_start(X[:, 3:6, :], x_r[:, 3:6, :]).then_inc(s_c1, 16)

    nc.gpsimd.memset(ones[:], 1.0)
    nc.gpsimd.memset(zerob[:], 0.0)
    nc.gpsimd.memset(wrhs[:], 0).then_inc(s_const, 1)

    nc.tensor.wait_ge(s_const, 1)
    for _ in range(WARMUP_MMS):
        nc.tensor.matmul(psw[:], ones[:], wrhs[:], start=True, stop=True)

    # ---- second block: compute + outputs (ACT table load lands here) ----
    for eng in nc.engines.values():
        eng.br("b2")
    nc.switch_body("b2")

    def sq_v(t):
        return nc.vector.scalar_tensor_tensor(
            out=xsq_v[:],
            in0=X[:, t, :],
            scalar=1.0,
            in1=X[:, t, :],
            op0=mult,
            op1=mult,
            accum_out=rowsq[:, t : t + 1],
        )

    def sq_a(t):
        return nc.scalar.activation(
            xsq_a[:],
            X[:, t, :],
            mybir.ActivationFunctionType.Square,
            bias=zerob[:],
            accum_out=rowsq[:, t : t + 1],
        )

    # DVE: tiles 0,1,2 (c0), 5 (c1), 6,7 (c2).  s_sqp>=2 (with ACT's
    # contribution) means tiles 0-5 are drained; s_sq>=1 means 6-7 are.
    nc.vector.wait_ge(s_c0, 16)
    sq_v(0)
    sq_v(1)
    sq_v(2)
    nc.vector.wait_ge(s_c1, 16)
    sq_v(5).then_inc(s_sqp, 1)
    nc.vector.wait_ge(s_c2, 16)
    sq_v(6)
    sq_v(7).then_inc(s_sq, 1)

    # ACT: tiles 3,4 (c1), then the two PSUM->SBUF copies and colsum out
    nc.scalar.wait_ge(s_const, 1)
    nc.scalar.wait_ge(s_c1, 16)
    sq_a(3)
    sq_a(4).then_inc(s_sqp, 1)
    nc.scalar.wait_ge(s_pe0, 1)
    nc.scalar.copy(cs[:, 0:D], ps0[:])
    nc.scalar.wait_ge(s_pe1, 1)
    nc.scalar.copy(cs[:, D : 2 * D], ps1[:])
    nc.scalar.dma_start(colsum_dram[:], cs[:]).then_inc(s_out1, 16)

    # PE: column-sum matmuls; bank0 = tiles 0-5, bank1 = tiles 6-7
    nc.tensor.wait_ge(s_c0, 16)
    nc.tensor.matmul(ps0[:], ones[:], X[:, 0, :], start=True, stop=False)
    nc.tensor.matmul(ps0[:], ones[:], X[:, 1, :], start=False, stop=False)
    nc.tensor.matmul(ps0[:], ones[:], X[:, 2, :], start=False, stop=False)
    nc.tensor.wait_ge(s_c1, 16)
    nc.tensor.matmul(ps0[:], ones[:], X[:, 3, :], start=False, stop=False)
    nc.tensor.matmul(ps0[:], ones[:], X[:, 4, :], start=False, stop=False)
    nc.tensor.matmul(
        ps0[:], ones[:], X[:, 5, :], start=False, stop=True
    ).then_inc(s_pe0, 1)
    nc.tensor.wait_ge(s_c2, 16)
    nc.tensor.matmul(ps1[:], ones[:], X[:, 6, :], start=True, stop=False)
    nc.tensor.matmul(
        ps1[:], ones[:], X[:, 7, :], start=False, stop=True
    ).then_inc(s_pe1, 1)

    # SP: rowsq leaves in two pieces so most of it overlaps the last
    # squares — tiles 0-5 as soon as they are drained (~0.9us before the
    # end), and only the 2-column remainder pays its DMA latency on the
    # tail.  Both increment s_out0 (reaches 32 when both have landed).
    nc.sync.wait_ge(s_sqp, 2)
    nc.sync.dma_start(rowsq_dram[:, 0:6], rowsq[:, 0:6]).then_inc(s_out0, 16)
    nc.sync.wait_ge(s_sq, 1)
    nc.sync.dma_start(rowsq_dram[:, 6:8], rowsq[:, 6:8]).then_inc(s_out0, 16)

    # The NEFF executes more than once per invocation (PJRT warmup), so
    # the semaphores must return to zero or the next execution's waits
    # all pass instantly.  Both output sems transitively dominate every
    # other wait, so gating the clear on them is sufficient; gpsimd is
    # the only waiter of the output sems, so the clear cannot race
    # another engine's pending wait.
    nc.gpsimd.wait_ge(s_out0, 32)
    nc.gpsimd.wait_ge(s_out1, 16)
    # One range-clear instead of per-sem clears: gpsimd is the last
    # engine to reach the wrapper's end barrier, so every instruction
    # here is on the measured critical path (~50ns each).
    all_sems = (
        s_const, s_c0, s_c1, s_c2, s_sqp, s_sq, s_pe0, s_pe1, s_out0, s_out1,
    )
    nums = sorted(s.num for s in all_sems)
    assert nums[-1] - nums[0] == len(nums) - 1, nums
    nc.gpsimd.sem_clear(range(nums[0], nums[-1] + 1))

    nc.compile()
    return nc


def get_nc():
    if not _nc_cache:
        _nc_cache.append(_build_nc())
    return _nc_cache[0]


def make_in_maps(x):
    x = np.ascontiguousarray(np.asarray(x), dtype=np.float32).astype(_bf16_np)
    return [{"x_blk": x[c * ROWS : (c + 1) * ROWS]} for c in range(NCORES)]


def combine_partials(rowsq_parts, colsum_parts):
    """rowsq_parts: per-core (P, T) row-squared-norm arrays; colsum_parts:
    per-core (1, 2*D) column-sum halves (psum banks 0 and 1) -> loss.
    Row order is irrelevant for sum/max, so no reindexing is needed."""
    S = 0.0
    maxsq = -np.inf
    for r in rowsq_parts:
        a = np.asarray(r, dtype=np.float64)
        S += a.sum()
        maxsq = max(maxsq, float(a.max()))
    s = np.zeros(D, dtype=np.float64)
    for c in colsum_parts:
        a = np.asarray(c, dtype=np.float64).reshape(-1)
        s += a[:D] + a[D:]
    count = N * (N - 1) // 2
    return np.float32((N * S - s @ s) / (np.sqrt(maxsq) * count))


def kernel(x):
    from concourse.bass_utils import run_bass_kernel_spmd

    nc = get_nc()
    in_maps = make_in_maps(x)

    def run_once():
        res = run_bass_kernel_spmd(nc, in_maps, list(range(NCORES)))
        return combine_partials(
            [r["rowsq"] for r in res.results],
            [r["colsum"] for r in res.results],
        )

    # The very first execution of a freshly loaded NEFF can inherit
    # non-zero semaphore state from the XLA helper NEFFs that staged the
    # inputs; every later execution starts from this kernel's own clean
    # end-state.  Run twice and return the settled result; if the two
    # disagree beyond noise, settle once more.
    prev, out = run_once(), run_once()
    if abs(float(out) - float(prev)) > 1e-3 * max(abs(float(out)), 1e-30):
        out = run_once()
    return out

